# revision 7
# baseline (speedup 1.0000x reference)
"""Grouped-experts SwiGLU MoE kernel for Trainium2 (8 NeuronCores).

Expert-parallel sharding: core e owns expert e's weights and its contiguous
token group (m_sizes gives T//E = 2048 tokens per expert). No collectives —
routing/scatter/gather happens on the host, each core runs an identical
single-core program on its own shard.

Per-core math: out = (silu(x_e @ w1_e) * (x_e @ w3_e)) @ w2_e
  x_e [2048, 2048], w1/w3 [2048, 1024], w2 [1024, 2048].

Device strategy (all matmul operands bf16: bf16 stationary gets
fast-weight-load — 97ns vs 187ns LDWEIGHTS — and bf16 moving halves the
SBUF stream bytes, so MMs issue at the 216ns/512-col floor):
  phase 1 (up+gate):  stationary = w1/w3 128x128 bf16 tiles, moving = xT
      tiles (bf16, pre-transposed on host so D is the partition/contraction
      axis). PSUM accumulates over D in f32; SwiGLU evac (ACT silu + DVE mul)
      writes the intermediate zT [H, M] as bf16.
  phase 2 (down):     stationary = zT 128x128 tiles (bf16), moving = w2
      tiles (bf16, resident in SBUF). PSUM accumulates over H; DVE copies to
      SBUF and DMA stores out [M, D] f32 in natural orientation.

All inputs are bf16 so everything fits in SBUF at once (xT 8MB + w1/w3 8MB +
w2 4MB + zT 2MB < 26MB): every input DMA is issued in one prologue, ordered
so the first matmul's tiles land first — no mid-stream DMA dependencies at
all.  Tokens are processed in two halves of 1024 so each of u/g gets 2 PSUM
banks double-buffered across h iterations (evac overlaps the next h's
matmuls).
"""

import numpy as np
import ml_dtypes

E, T, D, H = 8, 16384, 2048, 1024
M = T // E            # tokens per expert
P = 128
DC = D // P           # 16 contraction chunks (phase 1)
HC = H // P           # 8 contraction chunks (phase 2)
NHALF = 2
MH = M // NHALF       # 1024 tokens per half
NMOV = 512            # moving free dim / PSUM bank width (f32)
G = 8                 # d-chunks per weight-stream DMA (256KB bf16 transfers)

_CACHE = {}
LAST_RESULTS = None   # for test harnesses that want the profile


def _build_program():
    import concourse.bacc as bacc
    import concourse.bass as bass
    import concourse.mybir as mybir
    import concourse.tile as tile

    f32 = mybir.dt.float32
    bf16 = mybir.dt.bfloat16
    SILU = mybir.ActivationFunctionType.Silu

    nc = bacc.Bacc("TRN2", target_bir_lowering=False, debug=False)

    xT = nc.dram_tensor("xT", [D, M], bf16, kind="ExternalInput")
    w1r = nc.dram_tensor("w1r", [HC, DC // G, P, G, P], bf16, kind="ExternalInput")
    w3r = nc.dram_tensor("w3r", [HC, DC // G, P, G, P], bf16, kind="ExternalInput")
    w2r = nc.dram_tensor("w2r", [HC, P, D], bf16, kind="ExternalInput")
    out = nc.dram_tensor("out", [M, D], f32, kind="ExternalOutput")

    xT_t = xT.rearrange("(c p) m -> p c m", p=P)  # [P, DC, M]

    with tile.TileContext(nc) as tc:
        with (
            tc.tile_pool(name="xp", bufs=1) as xp,
            tc.tile_pool(name="wp", bufs=1) as wp,
            tc.tile_pool(name="w2p", bufs=1) as w2p,
            tc.tile_pool(name="zp", bufs=1) as zp,
            tc.tile_pool(name="op", bufs=2) as op,
            tc.tile_pool(name="sp", bufs=3) as sp,
            tc.tile_pool(name="ps", bufs=2, space=bass.MemorySpace.PSUM) as ps,
        ):
            xt = xp.tile([P, DC, M], bf16, tag="xt")         # 64KB/partition
            w1t = wp.tile([P, HC, DC, P], bf16, tag="w1")    # 32KB/partition
            w3t = wp.tile([P, HC, DC, P], bf16, tag="w3")    # 32KB/partition
            w2t = w2p.tile([P, HC, D], bf16, tag="w2")       # 32KB/partition

            # ---- prologue: every input DMA, on two parallel HWDGE rings ----
            # Sync ring carries the xt chunks (first matmul's moving operand
            # first); the Scalar ring carries all weights concurrently, first
            # h0/cg0 split in half so the first LDWEIGHTS waits for ~128KB.
            # Scalar ring: just the first matmuls' stationary tiles (4 small
            # issues, done in ~3us — anything more would queue ahead of the
            # ACT_TABLE_LOAD and phase-1 SILUs on the Scalar FIFO).
            nc.scalar.dma_start(w1t[:, 0, 0:4, :], w1r[0, 0, :, 0:4, :])
            nc.scalar.dma_start(w3t[:, 0, 0:4, :], w3r[0, 0, :, 0:4, :])
            nc.scalar.dma_start(w1t[:, 0, 4:8, :], w1r[0, 0, :, 4:8, :])
            nc.scalar.dma_start(w3t[:, 0, 4:8, :], w3r[0, 0, :, 4:8, :])
            # Sync ring: xt chunks in consumption order, with each later
            # weight tile slotted in just ahead of when phase 1 reaches it.
            for c in range(0, 4):
                nc.sync.dma_start(xt[:, c, 0:MH], xT_t[:, c, 0:MH])
            nc.sync.dma_start(w1t[:, 0, G:2 * G, :], w1r[0, 1])
            nc.sync.dma_start(w3t[:, 0, G:2 * G, :], w3r[0, 1])
            for c in range(4, DC):
                nc.sync.dma_start(xt[:, c, 0:MH], xT_t[:, c, 0:MH])
            for h in range(1, HC):
                for cg in range(DC // G):
                    nc.sync.dma_start(w1t[:, h, cg * G:(cg + 1) * G, :], w1r[h, cg])
                    nc.sync.dma_start(w3t[:, h, cg * G:(cg + 1) * G, :], w3r[h, cg])
            for h in range(HC):
                nc.sync.dma_start(w2t[:, h, :], w2r[h])
            for c in range(DC):
                nc.sync.dma_start(xt[:, c, MH:M], xT_t[:, c, MH:M])

            # ---- PE warm-up during the initial DMA wait ----
            # The HAM clock gate holds the PE at 1.2GHz until it has been
            # busy ~3.4us. Burn that window on dummy matmuls over a scratch
            # tile while the first real tiles are still in flight, so the
            # real stream starts at 2.4GHz. 8 cold 512-col MMs ~= 3.5us.
            warm = sp.tile([P, NMOV], bf16, tag="warm")
            nc.gpsimd.memset(warm[:], 0)
            pw = ps.tile([P, NMOV], f32, tag="p0", name="warm")
            for _ in range(8):
                nc.tensor.matmul(pw[:], warm[:, 0:P], warm[:],
                                 start=True, stop=True)

            for hf in range(NHALF):
                m0 = hf * MH
                zt = zp.tile([P, HC, MH], bf16, tag="zt")    # 16KB/partition

                # ---- phase 1: u = x@w1, g = x@w3, z = silu(u)*g ----
                for h in range(HC):
                    pu = [ps.tile([P, NMOV], f32, tag=f"p{i}", name=f"pu{i}") for i in range(2)]
                    pg = [ps.tile([P, NMOV], f32, tag=f"p{i + 2}", name=f"pg{i}") for i in range(2)]
                    for c in range(DC):
                        first, last = c == 0, c == DC - 1
                        for mi in range(MH // NMOV):
                            nc.tensor.matmul(
                                pu[mi][:], w1t[:, h, c, :],
                                xt[:, c, m0 + mi * NMOV:m0 + (mi + 1) * NMOV],
                                start=first, stop=last,
                            )
                        for mi in range(MH // NMOV):
                            nc.tensor.matmul(
                                pg[mi][:], w3t[:, h, c, :],
                                xt[:, c, m0 + mi * NMOV:m0 + (mi + 1) * NMOV],
                                start=first, stop=last,
                            )
                    for mi in range(MH // NMOV):
                        st = sp.tile([P, NMOV], f32, tag="st")
                        nc.scalar.activation(st[:], pu[mi][:], SILU)
                        nc.vector.tensor_mul(
                            zt[:, h, mi * NMOV:(mi + 1) * NMOV],
                            st[:], pg[mi][:],
                        )

                # ---- phase 2: out = z @ w2 ----
                # dd-outer so each PSUM bank finishes its h-accumulation
                # while the next starts; its copy + 256KB store overlap the
                # remaining matmuls, leaving a ~2us tail after the last MM.
                for mi in range(MH // P):
                    po = [ps.tile([P, NMOV], f32, tag=f"p{dd}", name=f"po{dd}") for dd in range(4)]
                    osb = op.tile([P, D], f32, tag="o")
                    r0 = m0 + mi * P
                    for dd in range(D // NMOV):
                        for h in range(HC):
                            nc.tensor.matmul(
                                po[dd][:], zt[:, h, mi * P:(mi + 1) * P],
                                w2t[:, h, dd * NMOV:(dd + 1) * NMOV],
                                start=h == 0, stop=h == HC - 1,
                            )
                        nc.vector.tensor_copy(
                            osb[:, dd * NMOV:(dd + 1) * NMOV], po[dd][:]
                        )
                        nc.sync.dma_start(
                            out[r0:r0 + P, dd * NMOV:(dd + 1) * NMOV],
                            osb[:, dd * NMOV:(dd + 1) * NMOV],
                        )

    nc.compile()
    return nc


def _get_program():
    if "nc" not in _CACHE:
        _CACHE["nc"] = _build_program()
    return _CACHE["nc"]


def _prep_w13(w):
    # [D, H] -> [HC, DC//G, P, G, P]; element [h,cg,p,g,m] = w[(cg*G+g)*P+p, h*P+m]
    return np.ascontiguousarray(
        w.reshape(DC // G, G, P, HC, P).transpose(3, 0, 2, 1, 4)
        .astype(ml_dtypes.bfloat16)
    )


def _numpy_fallback(x, w1, w2, w3, m_sizes):
    offs = np.concatenate([[0], np.cumsum(np.asarray(m_sizes, dtype=np.int64))])
    out = np.zeros((x.shape[0], w2.shape[2]), dtype=np.float32)
    for e in range(w1.shape[0]):
        xe = x[offs[e]:offs[e + 1]]
        u = xe @ w1[e]
        g = xe @ w3[e]
        z = (u / (1.0 + np.exp(-u))) * g
        out[offs[e]:offs[e + 1]] = z @ w2[e]
    return out


def kernel(x, w1, w2, w3, m_sizes, _trace=False, _trace_kwargs=None):
    global LAST_RESULTS
    x = np.ascontiguousarray(x, dtype=np.float32)
    w1 = np.ascontiguousarray(w1, dtype=np.float32)
    w2 = np.ascontiguousarray(w2, dtype=np.float32)
    w3 = np.ascontiguousarray(w3, dtype=np.float32)
    m = np.asarray(m_sizes, dtype=np.int64)

    expected = (
        x.shape == (T, D)
        and w1.shape == (E, D, H)
        and w2.shape == (E, H, D)
        and w3.shape == (E, D, H)
        and m.shape == (E,)
        and np.all(m == M)
    )
    if not expected:
        return _numpy_fallback(x, w1, w2, w3, m_sizes)

    from concourse.bass_utils import run_bass_kernel_spmd

    nc = _get_program()
    in_maps = []
    for e in range(E):
        in_maps.append({
            "xT": np.ascontiguousarray(
                x[e * M:(e + 1) * M].T.astype(ml_dtypes.bfloat16)
            ),
            "w1r": _prep_w13(w1[e]),
            "w3r": _prep_w13(w3[e]),
            "w2r": np.ascontiguousarray(
                w2[e].astype(ml_dtypes.bfloat16).reshape(HC, P, D)
            ),
        })

    res = run_bass_kernel_spmd(
        nc, in_maps, core_ids=list(range(E)),
        trace=_trace, **(_trace_kwargs or {}),
    )
    LAST_RESULTS = res
    return np.concatenate([r["out"] for r in res.results], axis=0)


# revision 8
# speedup vs baseline: 1.0076x; 1.0076x over previous
"""Grouped-experts SwiGLU MoE kernel for Trainium2 (8 NeuronCores).

Expert-parallel sharding: core e owns expert e's weights and its contiguous
token group (m_sizes gives T//E = 2048 tokens per expert). No collectives —
routing/scatter/gather happens on the host, each core runs an identical
single-core program on its own shard.

Per-core math: out = (silu(x_e @ w1_e) * (x_e @ w3_e)) @ w2_e
  x_e [2048, 2048], w1/w3 [2048, 1024], w2 [1024, 2048].

Device strategy (all matmul operands bf16: bf16 stationary gets
fast-weight-load — 97ns vs 187ns LDWEIGHTS — and bf16 moving halves the
SBUF stream bytes, so MMs issue at the 216ns/512-col floor):
  phase 1 (up+gate):  stationary = w1/w3 128x128 bf16 tiles, moving = xT
      tiles (bf16, pre-transposed on host so D is the partition/contraction
      axis). PSUM accumulates over D in f32; SwiGLU evac (ACT silu + DVE mul)
      writes the intermediate zT [H, M] as bf16.
  phase 2 (down):     stationary = zT 128x128 tiles (bf16), moving = w2
      tiles (bf16, resident in SBUF). PSUM accumulates over H; DVE copies to
      SBUF and DMA stores out [M, D] f32 in natural orientation.

All inputs are bf16 so everything fits in SBUF at once (xT 8MB + w1/w3 8MB +
w2 4MB + zT 2MB < 26MB): every input DMA is issued in one prologue, ordered
so the first matmul's tiles land first — no mid-stream DMA dependencies at
all.  Tokens are processed in two halves of 1024 so each of u/g gets 2 PSUM
banks double-buffered across h iterations (evac overlaps the next h's
matmuls).
"""

import numpy as np
import ml_dtypes

E, T, D, H = 8, 16384, 2048, 1024
M = T // E            # tokens per expert
P = 128
DC = D // P           # 16 contraction chunks (phase 1)
HC = H // P           # 8 contraction chunks (phase 2)
NHALF = 2
MH = M // NHALF       # 1024 tokens per half
NMOV = 512            # moving free dim / PSUM bank width (f32)
G = 8                 # d-chunks per weight-stream DMA (256KB bf16 transfers)

_CACHE = {}
LAST_RESULTS = None   # for test harnesses that want the profile


def _build_program():
    import concourse.bacc as bacc
    import concourse.bass as bass
    import concourse.mybir as mybir
    import concourse.tile as tile

    f32 = mybir.dt.float32
    bf16 = mybir.dt.bfloat16
    SILU = mybir.ActivationFunctionType.Silu

    nc = bacc.Bacc("TRN2", target_bir_lowering=False, debug=False)

    xT = nc.dram_tensor("xT", [D, M], bf16, kind="ExternalInput")
    w1r = nc.dram_tensor("w1r", [HC, DC // G, P, G, P], bf16, kind="ExternalInput")
    w3r = nc.dram_tensor("w3r", [HC, DC // G, P, G, P], bf16, kind="ExternalInput")
    w2r = nc.dram_tensor("w2r", [HC, P, D], bf16, kind="ExternalInput")
    out = nc.dram_tensor("out", [M, D], f32, kind="ExternalOutput")

    xT_t = xT.rearrange("(c p) m -> p c m", p=P)  # [P, DC, M]

    with tile.TileContext(nc) as tc:
        with (
            tc.tile_pool(name="xp", bufs=1) as xp,
            tc.tile_pool(name="wp", bufs=1) as wp,
            tc.tile_pool(name="w2p", bufs=1) as w2p,
            tc.tile_pool(name="zp", bufs=1) as zp,
            tc.tile_pool(name="op", bufs=2) as op,
            tc.tile_pool(name="sp", bufs=3) as sp,
            tc.tile_pool(name="ps", bufs=2, space=bass.MemorySpace.PSUM) as ps,
        ):
            xt = xp.tile([P, DC, M], bf16, tag="xt")         # 64KB/partition
            w1t = wp.tile([P, HC, DC, P], bf16, tag="w1")    # 32KB/partition
            w3t = wp.tile([P, HC, DC, P], bf16, tag="w3")    # 32KB/partition
            w2t = w2p.tile([P, HC, D], bf16, tag="w2")       # 32KB/partition

            # ---- prologue: every input DMA, on two parallel HWDGE rings ----
            # Sync ring carries the xt chunks (first matmul's moving operand
            # first); the Scalar ring carries all weights concurrently, first
            # h0/cg0 split in half so the first LDWEIGHTS waits for ~128KB.
            # Scalar ring: just the first matmuls' stationary tiles (4 small
            # issues, done in ~3us — anything more would queue ahead of the
            # ACT_TABLE_LOAD and phase-1 SILUs on the Scalar FIFO).
            nc.scalar.dma_start(w1t[:, 0, 0:4, :], w1r[0, 0, :, 0:4, :])
            nc.scalar.dma_start(w3t[:, 0, 0:4, :], w3r[0, 0, :, 0:4, :])
            nc.scalar.dma_start(w1t[:, 0, 4:8, :], w1r[0, 0, :, 4:8, :])
            nc.scalar.dma_start(w3t[:, 0, 4:8, :], w3r[0, 0, :, 4:8, :])
            # Sync ring: xt chunks in consumption order, with each later
            # weight tile slotted in just ahead of when phase 1 reaches it.
            for c in range(0, 4):
                nc.sync.dma_start(xt[:, c, 0:MH], xT_t[:, c, 0:MH])
            nc.sync.dma_start(w1t[:, 0, G:2 * G, :], w1r[0, 1])
            nc.sync.dma_start(w3t[:, 0, G:2 * G, :], w3r[0, 1])
            for c in range(4, DC):
                nc.sync.dma_start(xt[:, c, 0:MH], xT_t[:, c, 0:MH])
            for h in range(1, HC):
                for cg in range(DC // G):
                    nc.sync.dma_start(w1t[:, h, cg * G:(cg + 1) * G, :], w1r[h, cg])
                    nc.sync.dma_start(w3t[:, h, cg * G:(cg + 1) * G, :], w3r[h, cg])
            for h in range(HC):
                nc.sync.dma_start(w2t[:, h, :], w2r[h])
            for c in range(DC):
                nc.sync.dma_start(xt[:, c, MH:M], xT_t[:, c, MH:M])
            # (No PE warm-up matmuls: the cold 1.2GHz start paces the PE to
            # the early DMA ramp rate almost exactly — warming the clock
            # first just starves the stream on xt chunks and re-throttles.)

            for hf in range(NHALF):
                m0 = hf * MH
                zt = zp.tile([P, HC, MH], bf16, tag="zt")    # 16KB/partition

                # ---- phase 1: u = x@w1, g = x@w3, z = silu(u)*g ----
                for h in range(HC):
                    pu = [ps.tile([P, NMOV], f32, tag=f"p{i}", name=f"pu{i}") for i in range(2)]
                    pg = [ps.tile([P, NMOV], f32, tag=f"p{i + 2}", name=f"pg{i}") for i in range(2)]
                    for c in range(DC):
                        first, last = c == 0, c == DC - 1
                        for mi in range(MH // NMOV):
                            nc.tensor.matmul(
                                pu[mi][:], w1t[:, h, c, :],
                                xt[:, c, m0 + mi * NMOV:m0 + (mi + 1) * NMOV],
                                start=first, stop=last,
                            )
                        for mi in range(MH // NMOV):
                            nc.tensor.matmul(
                                pg[mi][:], w3t[:, h, c, :],
                                xt[:, c, m0 + mi * NMOV:m0 + (mi + 1) * NMOV],
                                start=first, stop=last,
                            )
                    for mi in range(MH // NMOV):
                        st = sp.tile([P, NMOV], f32, tag="st")
                        nc.scalar.activation(st[:], pu[mi][:], SILU)
                        nc.vector.tensor_mul(
                            zt[:, h, mi * NMOV:(mi + 1) * NMOV],
                            st[:], pg[mi][:],
                        )

                # ---- phase 2: out = z @ w2 ----
                # dd-outer so each PSUM bank finishes its h-accumulation
                # while the next starts; its copy + 256KB store overlap the
                # remaining matmuls, leaving a ~2us tail after the last MM.
                for mi in range(MH // P):
                    po = [ps.tile([P, NMOV], f32, tag=f"p{dd}", name=f"po{dd}") for dd in range(4)]
                    osb = op.tile([P, D], f32, tag="o")
                    r0 = m0 + mi * P
                    for dd in range(D // NMOV):
                        for h in range(HC):
                            nc.tensor.matmul(
                                po[dd][:], zt[:, h, mi * P:(mi + 1) * P],
                                w2t[:, h, dd * NMOV:(dd + 1) * NMOV],
                                start=h == 0, stop=h == HC - 1,
                            )
                        nc.vector.tensor_copy(
                            osb[:, dd * NMOV:(dd + 1) * NMOV], po[dd][:]
                        )
                        nc.sync.dma_start(
                            out[r0:r0 + P, dd * NMOV:(dd + 1) * NMOV],
                            osb[:, dd * NMOV:(dd + 1) * NMOV],
                        )

    nc.compile()
    return nc


def _get_program():
    if "nc" not in _CACHE:
        _CACHE["nc"] = _build_program()
    return _CACHE["nc"]


def _prep_w13(w):
    # [D, H] -> [HC, DC//G, P, G, P]; element [h,cg,p,g,m] = w[(cg*G+g)*P+p, h*P+m]
    return np.ascontiguousarray(
        w.reshape(DC // G, G, P, HC, P).transpose(3, 0, 2, 1, 4)
        .astype(ml_dtypes.bfloat16)
    )


def _numpy_fallback(x, w1, w2, w3, m_sizes):
    offs = np.concatenate([[0], np.cumsum(np.asarray(m_sizes, dtype=np.int64))])
    out = np.zeros((x.shape[0], w2.shape[2]), dtype=np.float32)
    for e in range(w1.shape[0]):
        xe = x[offs[e]:offs[e + 1]]
        u = xe @ w1[e]
        g = xe @ w3[e]
        z = (u / (1.0 + np.exp(-u))) * g
        out[offs[e]:offs[e + 1]] = z @ w2[e]
    return out


def kernel(x, w1, w2, w3, m_sizes, _trace=False, _trace_kwargs=None):
    global LAST_RESULTS
    x = np.ascontiguousarray(x, dtype=np.float32)
    w1 = np.ascontiguousarray(w1, dtype=np.float32)
    w2 = np.ascontiguousarray(w2, dtype=np.float32)
    w3 = np.ascontiguousarray(w3, dtype=np.float32)
    m = np.asarray(m_sizes, dtype=np.int64)

    expected = (
        x.shape == (T, D)
        and w1.shape == (E, D, H)
        and w2.shape == (E, H, D)
        and w3.shape == (E, D, H)
        and m.shape == (E,)
        and np.all(m == M)
    )
    if not expected:
        return _numpy_fallback(x, w1, w2, w3, m_sizes)

    from concourse.bass_utils import run_bass_kernel_spmd

    nc = _get_program()
    in_maps = []
    for e in range(E):
        in_maps.append({
            "xT": np.ascontiguousarray(
                x[e * M:(e + 1) * M].T.astype(ml_dtypes.bfloat16)
            ),
            "w1r": _prep_w13(w1[e]),
            "w3r": _prep_w13(w3[e]),
            "w2r": np.ascontiguousarray(
                w2[e].astype(ml_dtypes.bfloat16).reshape(HC, P, D)
            ),
        })

    res = run_bass_kernel_spmd(
        nc, in_maps, core_ids=list(range(E)),
        trace=_trace, **(_trace_kwargs or {}),
    )
    LAST_RESULTS = res
    return np.concatenate([r["out"] for r in res.results], axis=0)
